# revision 9
# baseline (speedup 1.0000x reference)
"""BEVCrossAttention (deformable attention) Trainium2 Bass kernel.

Strategy (8 NeuronCores, sharded over queries, value replicated):
- Per core: 1250 queries (padded to 1280 = 10 tiles of 128).
- Projections on TensorE (bf16 operands, f32 PSUM); biases folded into the
  matmul K-chain via a ones-row (K=1 matmul).
- v = value @ W_val written to DRAM head-major as bf16 rows of 32 channels;
  bilinear sampling fetches x-adjacent row PAIRS (64 B * 2 = 128 B blocks)
  with one indirect DMA gather per (head, query-tile): idx granularity is one
  row (coef=32 elems), each index copies 64 contiguous elems = rows (r, r+1).
- Corner weights computed branch-free with the tent formulation:
    bx = floor(clip(px, 0, W-2)); wx0 = relu(1-(px-bx)); wx1 = relu(min(t, 2-t))
  which reproduces grid_sample zero-padding semantics exactly (validated in
  numpy against the reference: f32 rel err 3.5e-7).
- Combine on VectorE: one broadcast multiply (weights x gathered patches) and
  one grouped reduce per (head, qtile); then output projection on TensorE.
"""

import numpy as np

import concourse.bass as bass
import concourse.mybir as mybir
from concourse.bass import IndirectOffsetOnAxis
from concourse.tile import TileContext

F32 = mybir.dt.float32
BF16 = mybir.dt.bfloat16
I32 = mybir.dt.int32

HEADS, LEVELS, POINTS, Z = 8, 4, 8, 4
E = 256
D = 32
SHAPES = [(92, 160), (46, 80), (23, 40), (12, 20)]
STARTS = [0, 14720, 18400, 19320]
NV = 19560
NVG = 19584  # head stride in v_dram (153 * 128)
NGT = 153    # value row tiles
NCORES = 8
NQ = 10000
QSH = 1250
QPAD = 1280
NQT = 10     # query tiles per core
NQB = 5      # query tiles per math batch
NBATCH = 2
TWO23 = 8388608.0

A = mybir.AluOpType
AF = mybir.ActivationFunctionType
AX = mybir.AxisListType


def build_nc(debug=False):
    nc = bass.Bass(trn_type="TRN2", target_bir_lowering=False)

    q_in = nc.dram_tensor("q", [QPAD, E], F32, kind="ExternalInput")
    qp_in = nc.dram_tensor("qp", [QPAD, E], F32, kind="ExternalInput")
    refp_in = nc.dram_tensor("refp", [QPAD, 2 * Z], F32, kind="ExternalInput")
    val_in = nc.dram_tensor("value", [NV, E], F32, kind="ExternalInput")
    woff_in = nc.dram_tensor("w_off", [E, 512], F32, kind="ExternalInput")
    boff_in = nc.dram_tensor("b_off", [1, 512], F32, kind="ExternalInput")
    wattn_in = nc.dram_tensor("w_attn", [E, 256], F32, kind="ExternalInput")
    battn_in = nc.dram_tensor("b_attn", [1, 256], F32, kind="ExternalInput")
    wval_in = nc.dram_tensor("w_val", [E, E], F32, kind="ExternalInput")
    bval_in = nc.dram_tensor("b_val", [1, E], F32, kind="ExternalInput")
    wout_in = nc.dram_tensor("w_out", [E, E], F32, kind="ExternalInput")
    bout_in = nc.dram_tensor("b_out", [1, E], F32, kind="ExternalInput")
    ones_in = nc.dram_tensor("ones", [1, 128], F32, kind="ExternalInput")
    out_dram = nc.dram_tensor("out", [QPAD, E], F32, kind="ExternalOutput")

    vdram = nc.dram_tensor("vdram", [HEADS * NVG, D], BF16, kind="Internal")
    if debug:
        dbg_off = nc.dram_tensor("dbg_off", [128, NQB, 512], F32, kind="ExternalOutput")
        dbg_attn = nc.dram_tensor("dbg_attn", [128, NQB, 256], F32, kind="ExternalOutput")
        dbg_idx = nc.dram_tensor("dbg_idx", [128, NQB, 512], F32, kind="ExternalOutput")
        dbg_w4 = nc.dram_tensor("dbg_w4", [128, NQB, 1024], F32, kind="ExternalOutput")
        dbg_acc = nc.dram_tensor("dbg_acc", [128, NQB, 256], F32, kind="ExternalOutput")
        dbg_g = nc.dram_tensor("dbg_g", [128, 64, 64], F32, kind="ExternalOutput")

    with TileContext(nc) as tc:
        with (
            tc.tile_pool(name="const", bufs=1) as cp,
            tc.tile_pool(name="wload", bufs=2) as wl,
            tc.tile_pool(name="vpipe", bufs=3) as vp,
            tc.tile_pool(name="batch", bufs=1) as bp,
            tc.tile_pool(name="mtmp", bufs=2) as mp,
            tc.tile_pool(name="gath", bufs=3) as gp,
            tc.tile_pool(name="oproj", bufs=2) as op_,
            tc.tile_pool(name="ps512", bufs=2, space="PSUM") as ps512,
            tc.tile_pool(name="ps256", bufs=2, space="PSUM") as ps256,
        ):
            # ---- constants / weights ----
            def load_cast(dram_ap, shape, name):
                t_f = wl.tile(shape, F32, tag="wtmp")
                nc.sync.dma_start(t_f[:], dram_ap)
                t_b = cp.tile(shape, BF16, tag=name)
                nc.vector.tensor_copy(t_b[:], t_f[:])
                return t_b

            woff_b = [load_cast(woff_in[eh * 128:(eh + 1) * 128, :], [128, 512], f"woff{eh}") for eh in range(2)]
            wattn_b = [load_cast(wattn_in[eh * 128:(eh + 1) * 128, :], [128, 256], f"wattn{eh}") for eh in range(2)]
            wval_b = [load_cast(wval_in[eh * 128:(eh + 1) * 128, :], [128, 256], f"wval{eh}") for eh in range(2)]
            wout_b = [load_cast(wout_in[eh * 128:(eh + 1) * 128, :], [128, 256], f"wout{eh}") for eh in range(2)]
            boff_b = load_cast(boff_in[:, :], [1, 512], "boff")
            battn_b = load_cast(battn_in[:, :], [1, 256], "battn")
            bval_b = load_cast(bval_in[:, :], [1, 256], "bval")
            bout_b = load_cast(bout_in[:, :], [1, 256], "bout")
            ones_b = load_cast(ones_in[:, :], [1, 128], "ones")

            qTb = cp.tile([128, 2, QPAD], BF16, tag="qTb")

            # ---- query + pos, transpose ----
            for qt in range(NQT):
                sl = slice(qt * 128, (qt + 1) * 128)
                qf = vp.tile([128, E], F32, tag="qld")
                nc.sync.dma_start(qf[:], q_in[sl, :])
                qpf = vp.tile([128, E], F32, tag="qpld")
                nc.sync.dma_start(qpf[:], qp_in[sl, :])
                qb = vp.tile([128, E], BF16, tag="qb")
                nc.vector.tensor_tensor(out=qb[:], in0=qf[:], in1=qpf[:], op=A.add)
                for eh in range(2):
                    nc.sync.dma_start_transpose(qTb[:, eh, sl], qb[:, eh * 128:(eh + 1) * 128])

            # ---- value projection -> vdram (head-major bf16) ----
            vdram_r = vdram[:].rearrange("(h g) c -> g h c", h=HEADS)
            for gt in range(NGT):
                rows = min(128, NV - gt * 128)
                vt = vp.tile([128, E], F32, tag="vld")
                if rows < 128:
                    nc.vector.memset(vt[:], 0.0)
                nc.sync.dma_start(vt[:rows, :], val_in[gt * 128: gt * 128 + rows, :])
                vb = vp.tile([128, E], BF16, tag="vb")
                nc.scalar.copy(vb[:], vt[:])
                vTb = vp.tile([128, 2, 128], BF16, tag="vTb")
                for eh in range(2):
                    nc.sync.dma_start_transpose(vTb[:, eh, :], vb[:, eh * 128:(eh + 1) * 128])
                vps = ps256.tile([128, 256], F32, tag="vps")
                nc.tensor.matmul(vps[:], vTb[:, 0, :], wval_b[0][:], start=True, stop=False)
                nc.tensor.matmul(vps[:], vTb[:, 1, :], wval_b[1][:], start=False, stop=False)
                nc.tensor.matmul(vps[:], ones_b[:], bval_b[:], start=False, stop=True)
                vsb = vp.tile([128, E], BF16, tag="vsb")
                nc.scalar.copy(vsb[:], vps[:])
                nc.sync.dma_start(
                    vdram_r[gt * 128:(gt + 1) * 128, :, :],
                    vsb[:].rearrange("g (h c) -> g h c", h=HEADS),
                )

            # ---- main batches ----
            for b in range(NBATCH):
                off_all = bp.tile([128, NQB, 512], F32, tag="off")
                eatt = bp.tile([128, NQB, 256], F32, tag="eatt")
                attn_all = bp.tile([128, NQB, 256], F32, tag="attn")
                refp_t = bp.tile([128, NQB, 2 * Z], F32, tag="refp")
                idxf = bp.tile([128, NQB, 512], F32, tag="idxf")
                idxi = bp.tile([128, NQB, 512], I32, tag="idxi")
                w4 = bp.tile([128, NQB, 1024], F32, tag="w4")
                accall = bp.tile([128, NQB, 256], F32, tag="acc")
                accb = bp.tile([128, NQB, 256], BF16, tag="accb")

                for i in range(NQB):
                    qt = b * NQB + i
                    sl = slice(qt * 128, (qt + 1) * 128)
                    nc.sync.dma_start(refp_t[:, i, :], refp_in[sl, :])
                    offps = ps512.tile([128, 512], F32, tag="offps")
                    nc.tensor.matmul(offps[:], qTb[:, 0, sl], woff_b[0][:], start=True, stop=False)
                    nc.tensor.matmul(offps[:], qTb[:, 1, sl], woff_b[1][:], start=False, stop=False)
                    nc.tensor.matmul(offps[:], ones_b[:], boff_b[:], start=False, stop=True)
                    nc.scalar.copy(off_all[:, i, :], offps[:])
                    attps = ps256.tile([128, 256], F32, tag="attps")
                    nc.tensor.matmul(attps[:], qTb[:, 0, sl], wattn_b[0][:], start=True, stop=False)
                    nc.tensor.matmul(attps[:], qTb[:, 1, sl], wattn_b[1][:], start=False, stop=False)
                    nc.tensor.matmul(attps[:], ones_b[:], battn_b[:], start=False, stop=True)
                    # softmax numerator straight out of PSUM (logits are small;
                    # max-subtraction unnecessary)
                    nc.scalar.activation(eatt[:, i, :], attps[:], AF.Exp)

                att_s = mp.tile([128, NQB * 8], F32, tag="atts")
                nc.vector.tensor_reduce(
                    att_s[:],
                    eatt[:].rearrange("q b (h lp) -> q (b h) lp", lp=LEVELS * POINTS),
                    axis=AX.X, op=A.add,
                )
                att_r = mp.tile([128, NQB * 8], F32, tag="attr")
                nc.vector.reciprocal(att_r[:], att_s[:])
                nc.vector.tensor_tensor(
                    out=attn_all[:].rearrange("q b (h lp) -> q (b h) lp", lp=32),
                    in0=eatt[:].rearrange("q b (h lp) -> q (b h) lp", lp=32),
                    in1=att_r[:].unsqueeze(2).broadcast_to([128, NQB * 8, 32]),
                    op=A.mult,
                )

                attn_r = attn_all[:].rearrange("q b (h l p) -> q b l h p", h=HEADS, l=LEVELS)
                idxf_r = idxf[:].rearrange("q b (h l p ky) -> q b l h p ky", h=HEADS, l=LEVELS, ky=2)
                w4_r = w4[:].rearrange("q b (h l p ky kx) -> q b l h p ky kx", h=HEADS, l=LEVELS, ky=2, kx=2)

                for l in range(LEVELS):
                    H_, W_ = SHAPES[l]
                    wgt = {}
                    flo = {}
                    for xy in range(2):  # 0 = x (width), 1 = y (height)
                        dim = W_ if xy == 0 else H_
                        osl = off_all[:, :, l * 128 + xy * 64: l * 128 + xy * 64 + 64]
                        p_ = mp.tile([128, NQB, 64], F32, tag="p_")
                        refw = mp.tile([128, NQB, Z], F32, tag="refw")
                        nc.vector.tensor_scalar(
                            out=refw[:], in0=refp_t[:, :, xy::2],
                            scalar1=float(dim), scalar2=-0.5, op0=A.mult, op1=A.add,
                        )
                        for pz in range(2):
                            nc.vector.tensor_tensor(
                                out=p_[:].rearrange("q b (h pz z) -> q b h pz z", h=HEADS, pz=2)[:, :, :, pz, :],
                                in0=osl.rearrange("q b (h pz z) -> q b h pz z", h=HEADS, pz=2)[:, :, :, pz, :],
                                in1=refw[:].unsqueeze(2).broadcast_to([128, NQB, HEADS, Z]),
                                op=A.add,
                            )
                        pc = mp.tile([128, NQB, 64], F32, tag="pc")
                        nc.vector.tensor_scalar(
                            out=pc[:], in0=p_[:], scalar1=float(dim - 2), scalar2=0.0,
                            op0=A.min, op1=A.max,
                        )
                        r_ = mp.tile([128, NQB, 64], F32, tag="r_")
                        nc.vector.tensor_scalar_add(r_[:], pc[:], TWO23)
                        nc.vector.tensor_scalar_add(r_[:], r_[:], -TWO23)
                        cmp = mp.tile([128, NQB, 64], F32, tag="cmp")
                        nc.vector.tensor_tensor(out=cmp[:], in0=r_[:], in1=pc[:], op=A.is_gt)
                        fl = mp.tile([128, NQB, 64], F32, tag=f"fl{xy}")
                        nc.vector.tensor_tensor(out=fl[:], in0=r_[:], in1=cmp[:], op=A.subtract)
                        t_ = mp.tile([128, NQB, 64], F32, tag="t_")
                        nc.vector.tensor_tensor(out=t_[:], in0=p_[:], in1=fl[:], op=A.subtract)
                        ta = mp.tile([128, NQB, 64], F32, tag="ta")
                        nc.vector.tensor_scalar(
                            out=ta[:], in0=t_[:], scalar1=0.0, scalar2=None, op0=A.abs_max,
                        )
                        wa = mp.tile([128, NQB, 64], F32, tag=f"wa{xy}")
                        nc.scalar.activation(wa[:], ta[:], AF.Relu, bias=1.0, scale=-1.0)
                        u_ = mp.tile([128, NQB, 64], F32, tag="u_")
                        nc.vector.tensor_scalar(
                            out=u_[:], in0=t_[:], scalar1=-1.0, scalar2=2.0, op0=A.mult, op1=A.add,
                        )
                        m_ = mp.tile([128, NQB, 64], F32, tag="m_")
                        nc.vector.tensor_tensor(out=m_[:], in0=t_[:], in1=u_[:], op=A.min)
                        wb = mp.tile([128, NQB, 64], F32, tag=f"wb{xy}")
                        nc.scalar.activation(wb[:], m_[:], AF.Relu)
                        wgt[(xy, 0)] = wa
                        wgt[(xy, 1)] = wb
                        flo[xy] = fl

                    gl = mp.tile([128, NQB, 64], F32, tag="gl")
                    nc.vector.tensor_scalar(
                        out=gl[:], in0=flo[1][:], scalar1=float(W_), scalar2=float(STARTS[l]),
                        op0=A.mult, op1=A.add,
                    )
                    nc.vector.tensor_tensor(out=gl[:], in0=gl[:], in1=flo[0][:], op=A.add)
                    glr = gl[:].rearrange("q b (h p) -> q b h p", h=HEADS)
                    nc.vector.tensor_copy(idxf_r[:, :, l, :, :, 0], glr)
                    nc.vector.tensor_scalar_add(idxf_r[:, :, l, :, :, 1], glr, float(W_))

                    # fold attn into the y-weights, then outer-product
                    asl = attn_r[:, :, l, :, :]
                    for ky in range(2):
                        wya = mp.tile([128, NQB, 64], F32, tag=f"wya{ky}")
                        nc.vector.tensor_tensor(
                            out=wya[:].rearrange("q b (h p) -> q b h p", h=HEADS),
                            in0=wgt[(1, ky)][:].rearrange("q b (h p) -> q b h p", h=HEADS),
                            in1=asl, op=A.mult,
                        )
                        for kx in range(2):
                            nc.vector.tensor_tensor(
                                out=w4_r[:, :, l, :, :, ky, kx],
                                in0=wya[:].rearrange("q b (h p) -> q b h p", h=HEADS),
                                in1=wgt[(0, kx)][:].rearrange("q b (h p) -> q b h p", h=HEADS),
                                op=A.mult,
                            )

                nc.vector.tensor_copy(idxi[:], idxf[:])

                if debug and b == 0:
                    nc.sync.dma_start(dbg_off[:], off_all[:])
                    nc.sync.dma_start(dbg_attn[:], attn_all[:])
                    nc.sync.dma_start(dbg_idx[:], idxf[:])
                    nc.sync.dma_start(dbg_w4[:], w4[:])

                # ---- gather + combine ----
                for i in range(NQB):
                    for h in range(HEADS):
                        gt_t = gp.tile([128, 64, 64], BF16, tag="G")
                        nc.gpsimd.indirect_dma_start(
                            out=gt_t[:],
                            out_offset=None,
                            in_=vdram[:],
                            in_offset=IndirectOffsetOnAxis(ap=idxi[:, i, h * 64:(h + 1) * 64], axis=0),
                            element_offset=h * NVG * D,
                        )
                        if debug and b == 0 and i == 0 and h == 0:
                            nc.gpsimd.dma_start(dbg_g[:], gt_t[:])
                        pm = gp.tile([128, 64, 2, 32], BF16, tag="P")
                        nc.vector.tensor_tensor(
                            out=pm[:],
                            in0=gt_t[:].rearrange("q lpk (kx c) -> q lpk kx c", kx=2),
                            in1=w4[:, i, h * 128:(h + 1) * 128]
                                .rearrange("q (lpk kx) -> q lpk kx", kx=2)
                                .unsqueeze(3).broadcast_to([128, 64, 2, 32]),
                            op=A.mult,
                        )
                        nc.vector.tensor_reduce(
                            accall[:, i, h * 32:(h + 1) * 32],
                            pm[:].rearrange("q lpk kx c -> q c (lpk kx)"),
                            axis=AX.X, op=A.add,
                        )

                nc.vector.tensor_copy(accb[:], accall[:])
                if debug and b == 0:
                    nc.sync.dma_start(dbg_acc[:], accall[:])

                # ---- output projection + residual ----
                for i in range(NQB):
                    qt = b * NQB + i
                    sl = slice(qt * 128, (qt + 1) * 128)
                    accT = op_.tile([128, 2, 128], BF16, tag="accT")
                    for ch in range(2):
                        nc.sync.dma_start_transpose(accT[:, ch, :], accb[:, i, ch * 128:(ch + 1) * 128])
                    ops = ps256.tile([128, 256], F32, tag="ops")
                    nc.tensor.matmul(ops[:], accT[:, 0, :], wout_b[0][:], start=True, stop=False)
                    nc.tensor.matmul(ops[:], accT[:, 1, :], wout_b[1][:], start=False, stop=False)
                    nc.tensor.matmul(ops[:], ones_b[:], bout_b[:], start=False, stop=True)
                    res = op_.tile([128, 256], F32, tag="res")
                    nc.sync.dma_start(res[:], q_in[sl, :])
                    osb = op_.tile([128, 256], F32, tag="osb")
                    nc.vector.tensor_tensor(out=osb[:], in0=ops[:], in1=res[:], op=A.add)
                    nc.sync.dma_start(out_dram[sl, :], osb[:])

    return nc


def make_in_maps(query, value, query_pos, reference_points,
                 W_off, b_off, W_attn, b_attn, W_val, b_val, W_out, b_out):
    # reorder W_off columns (h, l, p, xy) -> (l, xy, h, p)
    woff_r = np.ascontiguousarray(
        W_off.reshape(E, HEADS, LEVELS, POINTS, 2).transpose(0, 2, 4, 1, 3).reshape(E, 512)
    ).astype(np.float32)
    boff_r = np.ascontiguousarray(
        b_off.reshape(HEADS, LEVELS, POINTS, 2).transpose(1, 3, 0, 2).reshape(1, 512)
    ).astype(np.float32)

    shared = dict(
        value=np.ascontiguousarray(value[0]).astype(np.float32),
        w_off=woff_r,
        b_off=boff_r,
        w_attn=np.ascontiguousarray(W_attn).astype(np.float32),
        b_attn=np.ascontiguousarray(b_attn).reshape(1, 256).astype(np.float32),
        w_val=np.ascontiguousarray(W_val).astype(np.float32),
        b_val=np.ascontiguousarray(b_val).reshape(1, E).astype(np.float32),
        w_out=np.ascontiguousarray(W_out).astype(np.float32),
        b_out=np.ascontiguousarray(b_out).reshape(1, E).astype(np.float32),
        ones=np.ones((1, 128), np.float32),
    )

    in_maps = []
    for c in range(NCORES):
        s = c * QSH
        qs = np.zeros((QPAD, E), np.float32)
        qs[:QSH] = query[0, s:s + QSH]
        qps = np.zeros((QPAD, E), np.float32)
        qps[:QSH] = query_pos[0, s:s + QSH]
        rs = np.zeros((QPAD, 2 * Z), np.float32)
        rs[:QSH] = reference_points[0, s:s + QSH].reshape(QSH, 2 * Z)
        in_maps.append(dict(q=qs, qp=qps, refp=rs, **shared))
    return in_maps


_NC_CACHE = {}


def kernel(query, value, query_pos, reference_points, spatial_shapes,
           W_off, b_off, W_attn, b_attn, W_val, b_val, W_out, b_out):
    from concourse.bass_utils import run_bass_kernel_spmd

    query = np.asarray(query)
    in_maps = make_in_maps(
        np.asarray(query), np.asarray(value), np.asarray(query_pos),
        np.asarray(reference_points), np.asarray(W_off), np.asarray(b_off),
        np.asarray(W_attn), np.asarray(b_attn), np.asarray(W_val),
        np.asarray(b_val), np.asarray(W_out), np.asarray(b_out),
    )
    if "nc" not in _NC_CACHE:
        _NC_CACHE["nc"] = build_nc()
    nc = _NC_CACHE["nc"]
    res = run_bass_kernel_spmd(nc, in_maps, core_ids=list(range(NCORES)))
    out = np.concatenate([res.results[c]["out"][:QSH] for c in range(NCORES)], axis=0)
    return out[None].astype(np.float32)
